# revision 14
# baseline (speedup 1.0000x reference)
"""Trainium2 Bass kernel: 8-layer ternary (BitNet-1.58) dense transformer.

Model (per reference):
    h = embed[input_ids]                                  # (B=2, S=1024, H=2048)
    8x: y = h @ ternary(W_l)^T + b_l ; h = LN(y + h)*g+b  # H=2048
    h = LN(h)*final_g + final_b
    logits = h @ ternary(head_W)^T                        # (B, S, V=32000)

Sharding over 8 NeuronCores:
  - Layers: data-parallel over the 2048 tokens (256 tokens/core). Each core
    streams the full layer weights; no collectives.
  - Head: 8-way tensor-parallel over vocab (4000 vocab rows/core). Final
    hidden states are exchanged with two AllGathers (one per 128-token tile)
    of fp16 transposed activations; each core computes all 2048 tokens x its
    vocab shard. Local tokens are computed from SBUF while the gather runs;
    remote tokens stream from the gathered buffer using the runtime
    partition id for ring addressing.

Precision: ternary weights are scaled by an exact power of two (2^-e, e~6)
so they are EXACT in fp8e4; the compensating factor (s * 2^e ~= 1) is folded
into the activation cast. Matmuls run fp16 (activations, stationary) x
fp8 (weights, moving); PSUM accumulates fp32; the residual/LN path stays
fp32. Logits are written bf16 and upcast on the host.

Scheduling notes:
  - The PE runs matmuls ONLY. Activation transposes go through the DMA XBAR
    (dma_start(transpose=True), fp16, SBUF->SBUF, out[p, kt, tok] =
    in[tok, kt*128 + p] — verified against CoreSim's InstDmaTransposeAnt),
    alternating between the two HWDGE queues (sync/scalar) by tile parity.
  - Software pipelining by emission order: each tile's next-layer cast is
    emitted directly after its ln_finish, so the (strict-FIFO) vector engine
    runs it before the other tile's LN chain, and the XBAR transpose lands
    during the other tile's matmuls. Layer-0 transposed inputs come from the
    host ("h0T"), so the first matmul waits only on two small DMAs.
  - The final-LN chains/casts/stores have no PE content; the AllGathers
    trigger while layer-8 and local-head matmuls still run. Remote peers
    stream per (core, tile) with ring addressing.
  - The board power governor caps sustained near-100%-duty matmul streams
    at K=13/16 (~1.95 GHz, type-31 HAM events) with large run-to-run
    variance; minimizing total PE cycles is what matters, so transposes are
    kept off the PE even though the PE has idle slots for them.
"""

import os
import sys

import numpy as np

try:
    import concourse.bass as bass
except ImportError:  # grading container should have it on sys.path already
    sys.path.insert(0, "/opt/trn_rl_repo")
    import concourse.bass as bass

import ml_dtypes
import concourse.mybir as mybir
import concourse.tile as tile
from concourse import bacc
from concourse.bass import ts
from concourse.bass_utils import run_bass_kernel_spmd
from contextlib import ExitStack

F32 = mybir.dt.float32
BF16 = mybir.dt.bfloat16
FP16 = mybir.dt.float16
FP8 = mybir.dt.float8e4
AX = mybir.AxisListType
OP = mybir.AluOpType
AF = mybir.ActivationFunctionType
EPS = 1e-5

# Full-size problem config (B=2, S=1024 -> 2048 tokens).
CFG_FULL = dict(L=8, H=2048, NTOK=2048, NC=8, TT=2, VS=4000, NV=500, CH=512)


def build_nc(cfg, sigmas, head_sigma, use_gb):
    L, H, NTOK, NC, TT = cfg["L"], cfg["H"], cfg["NTOK"], cfg["NC"], cfg["TT"]
    VS, NV, CH = cfg["VS"], cfg["NV"], cfg["CH"]
    KT = H // 128
    KH = KT // 2  # k-tiles per weight half
    NCH = H // CH
    NVC = VS // NV  # vocab chunks (8)
    assert NTOK == NC * TT * 128

    nc = bacc.Bacc("TRN2", target_bir_lowering=False, debug=False, num_devices=NC)
    h0 = nc.declare_dram_parameter("h0", [TT, 128, H], F32, isOutput=False)
    h0T = nc.declare_dram_parameter("h0T", [TT, 128, KT, 128], FP16, isOutput=False)
    w_ = nc.declare_dram_parameter("w", [L, KT, 128, H], FP8, isOutput=False)
    if use_gb:
        lng = nc.declare_dram_parameter("lng", [L, H], BF16, isOutput=False)
        lnb = nc.declare_dram_parameter("lnb", [L, H], BF16, isOutput=False)
        lbias = nc.declare_dram_parameter("lbias", [L, H], BF16, isOutput=False)
        fing = nc.declare_dram_parameter("fing", [H], BF16, isOutput=False)
        finb = nc.declare_dram_parameter("finb", [H], BF16, isOutput=False)
    hw_ = nc.declare_dram_parameter("hw", [KT, 128, VS], FP8, isOutput=False)
    eps_d = nc.declare_dram_parameter("eps", [128, 1], F32, isOutput=False)
    out = nc.declare_dram_parameter("out", [NTOK, VS], BF16, isOutput=True)
    hT_loc = [nc.dram_tensor(f"hT_loc{t}", [128, H], FP16) for t in range(TT)]
    hT_all = [
        nc.dram_tensor(
            f"hT_all{t}",
            [NC, 128, H],
            FP16,
            addr_space="Shared" if NC > 4 else "Local",
        )
        for t in range(TT)
    ]

    pid = nc.partition_id()

    with tile.TileContext(nc) as tc:
        ccs = [None] * TT
        with ExitStack() as ctxTop:
            hwp = ctxTop.enter_context(tc.tile_pool(name="hw", bufs=1))
            hTfinp = ctxTop.enter_context(tc.tile_pool(name="hTfin", bufs=TT))
            outp = ctxTop.enter_context(tc.tile_pool(name="outstg", bufs=2))
            hwt = hwp.tile([128, KT, VS], FP8)
            hTfin = [None] * TT

            def head_unit(hT_src, row_slot, name, psum_pool, psum_tag):
                """One 128-token tile x all 4000 vocab of this core's shard."""
                pss = [
                    psum_pool.tile(
                        [128, 512], F32, tag=psum_tag, name=f"ph{name}_{v}"
                    )
                    for v in range(NVC)
                ]
                for kt in range(KT):
                    for vi in range(NVC):
                        nc.tensor.matmul(
                            pss[vi][:, 0:NV],
                            lhsT=hT_src[:, kt, :],
                            rhs=hwt[:, kt, vi * NV : (vi + 1) * NV],
                            start=(kt == 0),
                            stop=(kt == KT - 1),
                            skip_group_check=True,
                        )
                o_t = outp.tile([128, VS], BF16, tag="ostg", name=f"o{name}")
                for vi in range(NVC):
                    nc.scalar.copy(o_t[:, vi * NV : (vi + 1) * NV], pss[vi][:, 0:NV])
                nc.scalar.dma_start(out[ts(row_slot, 128), :], o_t[:])

            with ExitStack() as ctxA:
                consts = ctxA.enter_context(tc.tile_pool(name="consts", bufs=1))
                state = ctxA.enter_context(tc.tile_pool(name="state", bufs=3))
                zpool = ctxA.enter_context(tc.tile_pool(name="z", bufs=2))
                hscp = ctxA.enter_context(tc.tile_pool(name="hsc", bufs=2))
                hTp = ctxA.enter_context(tc.tile_pool(name="hT", bufs=2))
                wp = ctxA.enter_context(tc.tile_pool(name="w", bufs=3))
                gbp = None
                if use_gb:
                    gbp = ctxA.enter_context(tc.tile_pool(name="gb", bufs=2))
                smp = ctxA.enter_context(tc.tile_pool(name="small", bufs=16))
                psY = ctxA.enter_context(
                    tc.tile_pool(name="psY", bufs=2 * NCH, space="PSUM")
                )

                eps_t = consts.tile([128, 1], F32)
                nc.sync.dma_start(eps_t[:], eps_d[:])

                h_cur = []
                hT = []
                for t in range(TT):
                    st = state.tile([128, H], F32, name=f"hinit{t}", tag="state")
                    nc.scalar.dma_start(st[:], h0[t])
                    h_cur.append(st)
                    # layer-0 transposed input comes precomputed from the host
                    ht = hTp.tile([128, KT, 128], FP16, tag="hT", name=f"hTp{t}")
                    nc.sync.dma_start(ht[:], h0T[t])
                    hT.append(ht)

                def cast_transpose(src_f32, scale_imm, pool, name, t):
                    """h [128tok, H] f32 -> hT [128, KT, 128tok] fp16 * scale."""
                    hsc = hscp.tile([128, H], FP16, tag="hsc", name=f"hsc{name}")
                    nc.vector.tensor_scalar_mul(hsc[:], src_f32[:], float(scale_imm))
                    dst = pool.tile([128, KT, 128], FP16, tag="hT", name=f"hT{name}")
                    eng = nc.sync if t == 0 else nc.scalar
                    eng.dma_start(dst[:], hsc[:], transpose=True)
                    return dst

                def ln_finish(affine_src, S_ap, SS_ap, g_t, b_t, name):
                    S = smp.tile([128, 1], F32, tag="s0", name=f"S{name}")
                    SS = smp.tile([128, 1], F32, tag="s1", name=f"SS{name}")
                    nc.vector.tensor_reduce(S[:], S_ap, axis=AX.X, op=OP.add)
                    nc.vector.tensor_reduce(SS[:], SS_ap, axis=AX.X, op=OP.add)
                    negmean = smp.tile([128, 1], F32, tag="s2", name=f"nm{name}")
                    nc.vector.tensor_scalar_mul(negmean[:], S[:], -1.0 / H)
                    msq = smp.tile([128, 1], F32, tag="s3", name=f"msq{name}")
                    nc.vector.tensor_scalar_mul(msq[:], SS[:], 1.0 / H)
                    var = smp.tile([128, 1], F32, tag="s4", name=f"var{name}")
                    nc.vector.tensor_tensor(var[:], negmean[:], negmean[:], OP.mult)
                    nc.vector.tensor_tensor(var[:], msq[:], var[:], OP.subtract)
                    std = smp.tile([128, 1], F32, tag="s5", name=f"std{name}")
                    nc.scalar.activation(std[:], var[:], AF.Sqrt, bias=eps_t[:])
                    rstd = smp.tile([128, 1], F32, tag="s6", name=f"rstd{name}")
                    nc.vector.reciprocal(rstd[:], std[:])
                    hn = state.tile([128, H], F32, tag="state", name=f"h{name}")
                    nc.vector.tensor_scalar(
                        hn[:], affine_src[:], negmean[:], rstd[:], OP.add, OP.mult
                    )
                    if g_t is not None:
                        nc.vector.tensor_tensor(hn[:], hn[:], g_t[:], OP.mult)
                        nc.vector.tensor_tensor(hn[:], hn[:], b_t[:], OP.add)
                    return hn

                fg = fb = None
                if use_gb:
                    fg = gbp.tile([128, H], BF16, tag="g", name="gfin")
                    nc.scalar.dma_start(fg[:], fing[None, :].to_broadcast((128, H)))
                    fb = gbp.tile([128, H], BF16, tag="b", name="bfin")
                    nc.scalar.dma_start(fb[:], finb[None, :].to_broadcast((128, H)))

                def emit_final(t):
                    """Final LN -> head input + store + gather for tile t.

                    Pure vector/scalar/DMA content — runs in the shadow of
                    the following matmul units.
                    """
                    h8 = h_cur[t]
                    fsums = smp.tile([128, 2], F32, tag="fsums", name=f"smfin{t}")
                    nc.vector.tensor_reduce(
                        fsums[:, 0:1], h8[:], axis=AX.X, op=OP.add
                    )
                    fsq = zpool.tile([128, H], F32, tag="z", name=f"fsq{t}")
                    nc.vector.tensor_tensor(fsq[:], h8[:], h8[:], OP.mult)
                    nc.vector.tensor_reduce(
                        fsums[:, 1:2], fsq[:], axis=AX.X, op=OP.add
                    )
                    hfin = ln_finish(
                        h8, fsums[:, 0:1], fsums[:, 1:2], fg, fb, f"fin{t}"
                    )
                    hTfin[t] = cast_transpose(
                        hfin, head_sigma, hTfinp, f"fin{t}", t
                    )
                    st_i = nc.sync.dma_start(hT_loc[t][:], hTfin[t][:])
                    cc = nc.gpsimd.collective_compute(
                        "AllGather",
                        OP.bypass,
                        replica_groups=[list(range(NC))],
                        ins=[hT_loc[t][:]],
                        outs=[hT_all[t][:]],
                    )
                    tile.add_dep_helper(
                        cc.ins, st_i.ins, sync=True,
                        reason=f"gather{t} waits on hT store{t}",
                    )
                    ccs[t] = cc

                gbt = {}
                for l in range(L):
                    w_half = []
                    for hf in range(2):
                        wt = wp.tile([128, KH, H], FP8, tag="w", name=f"w{l}_{hf}")
                        nc.scalar.dma_start(
                            wt[:],
                            w_[l, hf * KH : (hf + 1) * KH].rearrange("k p o -> p k o"),
                        )
                        w_half.append(wt)
                    # stream head-weight chunks in the shadow of the layer loop
                    nc.scalar.dma_start(
                        hwt[:, 2 * l : 2 * l + 2, :],
                        hw_[2 * l : 2 * l + 2].rearrange("k p v -> p k v"),
                    )
                    if use_gb:
                        g_t = gbp.tile([128, H], BF16, tag="g", name=f"g{l}")
                        nc.scalar.dma_start(
                            g_t[:], lng[l][None, :].to_broadcast((128, H))
                        )
                        b_t = gbp.tile([128, H], BF16, tag="b", name=f"b{l}")
                        nc.scalar.dma_start(
                            b_t[:], lnb[l][None, :].to_broadcast((128, H))
                        )
                        bias_t = gbp.tile([128, H], BF16, tag="bias", name=f"bias{l}")
                        nc.scalar.dma_start(
                            bias_t[:], lbias[l][None, :].to_broadcast((128, H))
                        )
                        gbt = dict(g=g_t, b=b_t, bias=bias_t)

                    for t in range(TT):
                        hTt = hT[t]
                        ps = []
                        for i in range(NCH):
                            p = psY.tile(
                                [128, CH], F32, tag="psY", name=f"ps{l}_{t}_{i}"
                            )
                            ps.append(p)
                        for kt in range(KT):
                            wt = w_half[kt // KH]
                            for i in range(NCH):
                                nc.tensor.matmul(
                                    ps[i][:],
                                    lhsT=hTt[:, kt, :],
                                    rhs=wt[:, kt % KH, i * CH : (i + 1) * CH],
                                    start=(kt == 0),
                                    stop=(kt == KT - 1),
                                )
                        z = zpool.tile([128, H], F32, tag="z", name=f"z{l}_{t}")
                        sums = smp.tile(
                            [128, 1 + NCH], F32, tag="sums", name=f"sm{l}_{t}"
                        )
                        resid = h_cur[t]
                        if use_gb:
                            hb = zpool.tile([128, H], F32, tag="hb", name=f"hb{l}_{t}")
                            nc.vector.tensor_tensor(
                                hb[:], h_cur[t][:], gbt["bias"][:], OP.add
                            )
                            resid = hb
                        for i in range(NCH):
                            nc.vector.tensor_add(
                                z[:, i * CH : (i + 1) * CH],
                                ps[i][:],
                                resid[:, i * CH : (i + 1) * CH],
                            )
                        nc.vector.tensor_reduce(
                            sums[:, 0:1], z[:], axis=AX.X, op=OP.add
                        )
                        for i in range(NCH):
                            nc.scalar.activation(
                                ps[i][:],
                                z[:, i * CH : (i + 1) * CH],
                                AF.Square,
                                accum_out=sums[:, 1 + i : 2 + i],
                            )
                        h_cur[t] = ln_finish(
                            z, sums[:, 0:1], sums[:, 1 : 1 + NCH],
                            gbt.get("g"), gbt.get("b"), f"{l}_{t}",
                        )
                        if l + 1 < L:
                            # emitted NOW: the cast sits right behind this
                            # tile's LN in the vector FIFO and the XBAR runs
                            # during the other tile's matmuls
                            hT[t] = cast_transpose(
                                h_cur[t], sigmas[l + 1], hTp, f"{l + 1}_{t}", t
                            )
                        else:
                            emit_final(t)

                # local head units overlap the AllGathers (psY banks reused)
                head_unit(hTfin[0], pid * TT + 0, "loc0", psY, "psY")
                head_unit(hTfin[1], pid * TT + 1, "loc1", psY, "psY")

            with ExitStack() as ctxB:
                peerp = ctxB.enter_context(tc.tile_pool(name="peer", bufs=2))
                psH = ctxB.enter_context(
                    tc.tile_pool(name="psH", bufs=NVC, space="PSUM")
                )

                # remote tokens: ring order c = (pid + r) & (NC-1)
                for r in range(1, NC):
                    c = (pid + r) & (NC - 1)
                    for t in range(TT):
                        pt = peerp.tile(
                            [128, KT, 128], FP16, tag="peer", name=f"peer{r}_{t}"
                        )
                        ld = nc.sync.dma_start(pt[:], hT_all[t][ts(c, 1), :, :])
                        tile.add_dep_helper(
                            ld.ins,
                            ccs[t].ins,
                            sync=True,
                            reason=f"peer load r{r} t{t} waits gather{t}",
                        )
                        head_unit(pt, c * TT + t, f"r{r}_{t}", psH, "ph")

    return nc


def _ternary(wmat):
    """Exact {-1,0,1} ternary tensor + fp32 scale, matching the reference."""
    w = np.asarray(wmat, dtype=np.float32)
    s = np.mean(np.abs(w), dtype=np.float32)
    t = np.clip(np.rint(w / (s + np.float32(1e-8))), -1.0, 1.0).astype(np.float32)
    return t, float(s)


def _split_scale(s):
    """s = sigma * 2^-e with sigma ~ 1 and 2^-e exact in fp8e4."""
    e = int(np.clip(np.round(-np.log2(s)), -7, 9))
    return s * (2.0**e), e


_NC_CACHE = {}
_LAST_RESULTS = None


def kernel(**inputs):
    global _LAST_RESULTS
    cfg = CFG_FULL
    L, H, NTOK, NC, TT, VS = (
        cfg["L"], cfg["H"], cfg["NTOK"], cfg["NC"], cfg["TT"], cfg["VS"],
    )
    KT = H // 128
    TPC = TT * 128  # tokens per core
    BF = ml_dtypes.bfloat16
    F8 = ml_dtypes.float8_e4m3fn

    ids = np.asarray(inputs["input_ids"]).astype(np.int64).reshape(-1)
    embed = np.asarray(inputs["embed"], dtype=np.float32)
    layer_w = np.asarray(inputs["layer_w"], dtype=np.float32)
    layer_b = np.asarray(inputs["layer_b"], dtype=np.float32)
    ln_g = np.asarray(inputs["ln_g"], dtype=np.float32)
    ln_b = np.asarray(inputs["ln_b"], dtype=np.float32)
    final_g = np.asarray(inputs["final_g"], dtype=np.float32)
    final_b = np.asarray(inputs["final_b"], dtype=np.float32)
    head_w = np.asarray(inputs["head_w"], dtype=np.float32)

    use_gb = not (
        np.all(layer_b == 0.0)
        and np.all(ln_g == 1.0)
        and np.all(ln_b == 0.0)
        and np.all(final_g == 1.0)
        and np.all(final_b == 0.0)
    )

    h0_full = embed[ids]  # [NTOK, H] fp32

    sigmas = []
    wT = np.empty([L, KT, 128, H], dtype=F8)
    for l in range(L):
        t, s = _ternary(layer_w[l])
        sig, e = _split_scale(s)
        sigmas.append(sig)
        wT[l] = (
            (np.ascontiguousarray(t.T) * np.float32(2.0**-e))
            .reshape(KT, 128, H)
            .astype(F8)
        )
    th, head_scale = _ternary(head_w)
    head_sigma, e_h = _split_scale(head_scale)
    headT = (
        (np.ascontiguousarray(th.T) * np.float32(2.0**-e_h))
        .reshape(KT, 128, -1)
        .astype(F8)
    )  # [KT, 128, V]

    key = (id(cfg), tuple(sigmas), head_sigma, use_gb)
    if key not in _NC_CACHE:
        _NC_CACHE.clear()
        nc = build_nc(cfg, sigmas, head_sigma, use_gb)
        # Bacc.finalize runs the TRN2 legalization passes (1-wait-per-
        # instruction event-semaphore split, matmul->ldweights wait motion,
        # register allocation). The PJRT exec path serializes nc as-is.
        nc.finalize()
        _NC_CACHE[key] = nc
    nc = _NC_CACHE[key]

    common = {
        "w": wT,
        "eps": np.full((128, 1), EPS, np.float32),
    }
    if use_gb:
        common.update(
            lng=ln_g.astype(BF),
            lnb=ln_b.astype(BF),
            lbias=layer_b.astype(BF),
            fing=final_g.astype(BF),
            finb=final_b.astype(BF),
        )
    in_maps = []
    for c in range(NC):
        h0c = np.ascontiguousarray(
            h0_full[c * TPC : (c + 1) * TPC].reshape(TT, 128, H)
        )
        # layer-0 transposed+scaled input, in XBAR block layout
        h0t = np.ascontiguousarray(
            (h0c * np.float32(sigmas[0]))
            .transpose(0, 2, 1)  # [TT, H, 128tok]
            .reshape(TT, KT, 128, 128)
            .transpose(0, 2, 1, 3)  # [TT, 128p, KT, 128tok]
        ).astype(np.float16)
        in_maps.append(
            dict(
                common,
                h0=h0c,
                h0T=h0t,
                hw=np.ascontiguousarray(headT[:, :, c * VS : (c + 1) * VS]),
            )
        )

    trace = bool(int(os.environ.get("TRIKERNEL_TRACE", "0")))
    res = run_bass_kernel_spmd(nc, in_maps, core_ids=list(range(NC)), trace=trace)
    _LAST_RESULTS = res

    full = np.concatenate(
        [np.asarray(res.results[c]["out"]) for c in range(NC)], axis=1
    )  # [NTOK, V] bf16
    return full.reshape(2, 1024, 32000).astype(np.float32)


# revision 21
# speedup vs baseline: 1.1092x; 1.1092x over previous
"""Trainium2 Bass kernel: 8-layer ternary (BitNet-1.58) dense transformer.

Model (per reference):
    h = embed[input_ids]                                  # (B=2, S=1024, H=2048)
    8x: y = h @ ternary(W_l)^T + b_l ; h = LN(y + h)*g+b  # H=2048
    h = LN(h)*final_g + final_b
    logits = h @ ternary(head_W)^T                        # (B, S, V=32000)

Sharding over 8 NeuronCores:
  - Layers: data-parallel over the 2048 tokens (256 tokens/core). Each core
    streams the full layer weights; no collectives.
  - Head: 8-way tensor-parallel over vocab (4000 vocab rows/core). Final
    hidden states are exchanged with two AllGathers (one per 128-token tile)
    of fp16 transposed activations; each core computes all 2048 tokens x its
    vocab shard. Local tokens are computed from SBUF while the gather runs;
    remote tokens stream from the gathered buffer using the runtime
    partition id for ring addressing.

Precision: ternary weights are scaled by an exact power of two (2^-e, e~6)
so they are EXACT in fp8e4; the compensating factor (s * 2^e ~= 1) is folded
into the activation cast. Matmuls run fp16 (activations, stationary) x
fp8 (weights, moving); PSUM accumulates fp32; the residual/LN path stays
fp32. Logits are written bf16 and upcast on the host.

Scheduling notes:
  - The PE runs matmuls ONLY. Activation transposes go through the DMA XBAR
    (dma_start(transpose=True), fp16, SBUF->SBUF, out[p, kt, tok] =
    in[tok, kt*128 + p] — verified against CoreSim's InstDmaTransposeAnt),
    alternating between the two HWDGE queues (sync/scalar) by tile parity.
  - Software pipelining by emission order: each tile's next-layer cast is
    emitted directly after its ln_finish, so the (strict-FIFO) vector engine
    runs it before the other tile's LN chain, and the XBAR transpose lands
    during the other tile's matmuls. Layer-0 transposed inputs come from the
    host ("h0T"), so the first matmul waits only on two small DMAs.
  - The final-LN chains/casts/stores have no PE content; the AllGathers
    trigger while layer-8 and local-head matmuls still run. Remote peers
    stream per (core, tile) with ring addressing.
  - The board power governor caps sustained near-100%-duty matmul streams
    at K=13/16 (~1.95 GHz, type-31 HAM events) with large run-to-run
    variance; minimizing total PE cycles is what matters, so transposes are
    kept off the PE even though the PE has idle slots for them.
"""

import os
import sys

import numpy as np

try:
    import concourse.bass as bass
except ImportError:  # grading container should have it on sys.path already
    sys.path.insert(0, "/opt/trn_rl_repo")
    import concourse.bass as bass

import ml_dtypes
import concourse.mybir as mybir
import concourse.tile as tile
from concourse import bacc
from concourse.bass import ts
from concourse.bass_utils import run_bass_kernel_spmd
from contextlib import ExitStack

F32 = mybir.dt.float32
BF16 = mybir.dt.bfloat16
FP16 = mybir.dt.float16
FP8 = mybir.dt.float8e4
AX = mybir.AxisListType
OP = mybir.AluOpType
AF = mybir.ActivationFunctionType
EPS = 1e-5

# Full-size problem config (B=2, S=1024 -> 2048 tokens).
CFG_FULL = dict(L=8, H=2048, NTOK=2048, NC=8, TT=2, VS=4000, NV=500, CH=512)


def build_nc(cfg, sigmas, head_sigma, use_gb):
    L, H, NTOK, NC, TT = cfg["L"], cfg["H"], cfg["NTOK"], cfg["NC"], cfg["TT"]
    VS, NV, CH = cfg["VS"], cfg["NV"], cfg["CH"]
    KT = H // 128
    KH = KT // 2  # k-tiles per weight half
    NCH = H // CH
    NVC = VS // NV  # vocab chunks (8)
    assert NTOK == NC * TT * 128

    nc = bacc.Bacc("TRN2", target_bir_lowering=False, debug=False, num_devices=NC)
    h0 = nc.declare_dram_parameter("h0", [TT, 128, H], F32, isOutput=False)
    h0T = nc.declare_dram_parameter("h0T", [TT, 128, KT, 128], FP16, isOutput=False)
    w_ = nc.declare_dram_parameter("w", [L, KT, 128, H], FP8, isOutput=False)
    if use_gb:
        lng = nc.declare_dram_parameter("lng", [L, H], BF16, isOutput=False)
        lnb = nc.declare_dram_parameter("lnb", [L, H], BF16, isOutput=False)
        lbias = nc.declare_dram_parameter("lbias", [L, H], BF16, isOutput=False)
        fing = nc.declare_dram_parameter("fing", [H], BF16, isOutput=False)
        finb = nc.declare_dram_parameter("finb", [H], BF16, isOutput=False)
    hw_ = nc.declare_dram_parameter("hw", [KT, 128, VS], FP8, isOutput=False)
    ident_d = nc.declare_dram_parameter("ident", [128, 128], F32, isOutput=False)
    eps_d = nc.declare_dram_parameter("eps", [128, 1], F32, isOutput=False)
    out = nc.declare_dram_parameter("out", [NTOK, VS], BF16, isOutput=True)
    hT_loc = [nc.dram_tensor(f"hT_loc{t}", [128, H], FP16) for t in range(TT)]
    hT_all = [
        nc.dram_tensor(
            f"hT_all{t}",
            [NC, 128, H],
            FP16,
            addr_space="Shared" if NC > 4 else "Local",
        )
        for t in range(TT)
    ]

    pid = nc.partition_id()

    with tile.TileContext(nc) as tc:
        ccs = [None] * TT
        with ExitStack() as ctxTop:
            hwp = ctxTop.enter_context(tc.tile_pool(name="hw", bufs=1))
            hTfinp = ctxTop.enter_context(tc.tile_pool(name="hTfin", bufs=TT))
            outp = ctxTop.enter_context(tc.tile_pool(name="outstg", bufs=2))
            hwt = hwp.tile([128, KT, VS], FP8)
            hTfin = [None] * TT

            def head_unit(hT_src, row_slot, name, psum_pool, psum_tag):
                """One 128-token tile x all 4000 vocab of this core's shard."""
                pss = [
                    psum_pool.tile(
                        [128, 512], F32, tag=psum_tag, name=f"ph{name}_{v}"
                    )
                    for v in range(NVC)
                ]
                for kt in range(KT):
                    for vi in range(NVC):
                        nc.tensor.matmul(
                            pss[vi][:, 0:NV],
                            lhsT=hT_src[:, kt, :],
                            rhs=hwt[:, kt, vi * NV : (vi + 1) * NV],
                            start=(kt == 0),
                            stop=(kt == KT - 1),
                            skip_group_check=True,
                        )
                o_t = outp.tile([128, VS], BF16, tag="ostg", name=f"o{name}")
                for vi in range(NVC):
                    nc.scalar.copy(o_t[:, vi * NV : (vi + 1) * NV], pss[vi][:, 0:NV])
                nc.scalar.dma_start(out[ts(row_slot, 128), :], o_t[:])

            with ExitStack() as ctxA:
                consts = ctxA.enter_context(tc.tile_pool(name="consts", bufs=1))
                state = ctxA.enter_context(tc.tile_pool(name="state", bufs=3))
                zpool = ctxA.enter_context(tc.tile_pool(name="z", bufs=2))
                hscp = ctxA.enter_context(tc.tile_pool(name="hsc", bufs=2))
                hTp = ctxA.enter_context(tc.tile_pool(name="hT", bufs=2))
                wp = ctxA.enter_context(tc.tile_pool(name="w", bufs=3))
                gbp = None
                if use_gb:
                    gbp = ctxA.enter_context(tc.tile_pool(name="gb", bufs=2))
                smp = ctxA.enter_context(tc.tile_pool(name="small", bufs=16))
                psY = ctxA.enter_context(
                    tc.tile_pool(name="psY", bufs=2 * NCH, space="PSUM")
                )

                eps_t = consts.tile([128, 1], F32)
                nc.sync.dma_start(eps_t[:], eps_d[:])
                ident = consts.tile([128, 128], F32)
                nc.sync.dma_start(ident[:], ident_d[:])

                # layer-0 weights go out on the scalar queue FIRST — the
                # first matmul waits only on these and the (small) h0T loads
                w_half0 = []
                for hf in range(2):
                    wt = wp.tile([128, KH, H], FP8, tag="w", name=f"w0_{hf}")
                    nc.scalar.dma_start(
                        wt[:], w_[0, hf * KH : (hf + 1) * KH].rearrange("k p o -> p k o")
                    )
                    w_half0.append(wt)
                nc.scalar.dma_start(
                    hwt[:, 0:2, :], hw_[0:2].rearrange("k p v -> p k v")
                )

                h_cur = []
                hT = []
                for t in range(TT):
                    # layer-0 transposed input comes precomputed from the host
                    ht = hTp.tile([128, KT, 128], FP16, tag="hT", name=f"hTp{t}")
                    nc.sync.dma_start(ht[:], h0T[t])
                    hT.append(ht)
                for t in range(TT):
                    st = state.tile([128, H], F32, name=f"hinit{t}", tag="state")
                    nc.scalar.dma_start(st[:], h0[t])
                    h_cur.append(st)

                def cast_transpose(src_f32, scale_imm, pool, name, t, eng=None):
                    """h [128tok, H] f32 -> hT [128, KT, 128tok] fp16 * scale.

                    Returns (dst_tile, xbar_instruction).
                    """
                    hsc = hscp.tile([128, H], FP16, tag="hsc", name=f"hsc{name}")
                    nc.vector.tensor_scalar_mul(hsc[:], src_f32[:], float(scale_imm))
                    dst = pool.tile([128, KT, 128], FP16, tag="hT", name=f"hT{name}")
                    if eng is None:
                        eng = nc.sync if t == 0 else nc.scalar
                    xb = eng.dma_start(dst[:], hsc[:], transpose=True)
                    return dst, xb

                def pe_transpose(src_f32, scale_imm, pool, name):
                    """PE-based transpose via psY chunks — used while the
                    bootstrap barrier owns the DMA rings (XBAR would block).
                    """
                    dst = pool.tile([128, KT, 128], FP16, tag="hT", name=f"hT{name}")
                    for j in range(NCH):
                        pc = psY.tile([128, CH], F32, tag="psY", name=f"pT{name}_{j}")
                        for u in range(CH // 128):
                            kt = j * (CH // 128) + u
                            nc.tensor.transpose(
                                pc[:, u * 128 : (u + 1) * 128],
                                src_f32[:, kt * 128 : (kt + 1) * 128],
                                ident[:],
                            )
                        nc.scalar.activation(
                            dst[:, j * (CH // 128) : (j + 1) * (CH // 128), :],
                            pc[:],
                            AF.Copy,
                            scale=float(scale_imm),
                        )
                    return dst

                def ln_finish(affine_src, S_ap, SS_ap, g_t, b_t, name):
                    S = smp.tile([128, 1], F32, tag="s0", name=f"S{name}")
                    SS = smp.tile([128, 1], F32, tag="s1", name=f"SS{name}")
                    nc.vector.tensor_reduce(S[:], S_ap, axis=AX.X, op=OP.add)
                    nc.vector.tensor_reduce(SS[:], SS_ap, axis=AX.X, op=OP.add)
                    negmean = smp.tile([128, 1], F32, tag="s2", name=f"nm{name}")
                    nc.vector.tensor_scalar_mul(negmean[:], S[:], -1.0 / H)
                    msq = smp.tile([128, 1], F32, tag="s3", name=f"msq{name}")
                    nc.vector.tensor_scalar_mul(msq[:], SS[:], 1.0 / H)
                    var = smp.tile([128, 1], F32, tag="s4", name=f"var{name}")
                    nc.vector.tensor_tensor(var[:], negmean[:], negmean[:], OP.mult)
                    nc.vector.tensor_tensor(var[:], msq[:], var[:], OP.subtract)
                    std = smp.tile([128, 1], F32, tag="s5", name=f"std{name}")
                    nc.scalar.activation(std[:], var[:], AF.Sqrt, bias=eps_t[:])
                    rstd = smp.tile([128, 1], F32, tag="s6", name=f"rstd{name}")
                    nc.vector.reciprocal(rstd[:], std[:])
                    hn = state.tile([128, H], F32, tag="state", name=f"h{name}")
                    nc.vector.tensor_scalar(
                        hn[:], affine_src[:], negmean[:], rstd[:], OP.add, OP.mult
                    )
                    if g_t is not None:
                        nc.vector.tensor_tensor(hn[:], hn[:], g_t[:], OP.mult)
                        nc.vector.tensor_tensor(hn[:], hn[:], b_t[:], OP.add)
                    return hn

                fg = fb = None
                if use_gb:
                    fg = gbp.tile([128, H], BF16, tag="g", name="gfin")
                    nc.scalar.dma_start(fg[:], fing[None, :].to_broadcast((128, H)))
                    fb = gbp.tile([128, H], BF16, tag="b", name="bfin")
                    nc.scalar.dma_start(fb[:], finb[None, :].to_broadcast((128, H)))

                def emit_final(t):
                    """Final LN -> head input + store for tile t.

                    Pure vector/scalar/DMA content — runs in the shadow of
                    the surrounding matmul units. For t=0 the SS sums run on
                    the scalar engine (psY scratch); for t=1 on the vector
                    engine (the banks are owned by the local head unit then).
                    """
                    h8 = h_cur[t]
                    if t == 0:
                        fsums = smp.tile(
                            [128, 1 + NCH], F32, tag="fsums", name=f"smfin{t}"
                        )
                        nc.vector.tensor_reduce(
                            fsums[:, 0:1], h8[:], axis=AX.X, op=OP.add
                        )
                        for i in range(NCH):
                            dump = psY.tile(
                                [128, CH], F32, tag="psY", name=f"dmp{t}_{i}"
                            )
                            nc.scalar.activation(
                                dump[:],
                                h8[:, i * CH : (i + 1) * CH],
                                AF.Square,
                                accum_out=fsums[:, 1 + i : 2 + i],
                            )
                        hfin = ln_finish(
                            h8, fsums[:, 0:1], fsums[:, 1 : 1 + NCH],
                            fg, fb, f"fin{t}",
                        )
                    else:
                        fsums = smp.tile(
                            [128, 2], F32, tag="fsums1", name=f"smfin{t}"
                        )
                        nc.vector.tensor_reduce(
                            fsums[:, 0:1], h8[:], axis=AX.X, op=OP.add
                        )
                        fsq = zpool.tile([128, H], F32, tag="z", name=f"fsq{t}")
                        nc.vector.tensor_tensor(fsq[:], h8[:], h8[:], OP.mult)
                        nc.vector.tensor_reduce(
                            fsums[:, 1:2], fsq[:], axis=AX.X, op=OP.add
                        )
                        hfin = ln_finish(
                            h8, fsums[:, 0:1], fsums[:, 1:2], fg, fb, f"fin{t}"
                        )
                    hTfin[t], xb = cast_transpose(
                        hfin, head_sigma, hTfinp, f"fin{t}", t, eng=nc.sync
                    )
                    st_i = nc.sync.dma_start(hT_loc[t][:], hTfin[t][:])
                    return st_i, xb

                EARLY_PE = 3  # layers whose input transpose falls inside the
                # bootstrap-barrier window: XBAR would block on the DMA rings
                finals = [None] * TT
                pending = [None, None]
                gbt = {}
                for l in range(L):
                    if l > 0:
                        w_half = []
                        for hf in range(2):
                            wt = wp.tile(
                                [128, KH, H], FP8, tag="w", name=f"w{l}_{hf}"
                            )
                            nc.scalar.dma_start(
                                wt[:],
                                w_[l, hf * KH : (hf + 1) * KH].rearrange(
                                    "k p o -> p k o"
                                ),
                            )
                            w_half.append(wt)
                        nc.scalar.dma_start(
                            hwt[:, 2 * l : 2 * l + 2, :],
                            hw_[2 * l : 2 * l + 2].rearrange("k p v -> p k v"),
                        )
                    else:
                        w_half = w_half0
                    if use_gb:
                        g_t = gbp.tile([128, H], BF16, tag="g", name=f"g{l}")
                        nc.scalar.dma_start(
                            g_t[:], lng[l][None, :].to_broadcast((128, H))
                        )
                        b_t = gbp.tile([128, H], BF16, tag="b", name=f"b{l}")
                        nc.scalar.dma_start(
                            b_t[:], lnb[l][None, :].to_broadcast((128, H))
                        )
                        bias_t = gbp.tile([128, H], BF16, tag="bias", name=f"bias{l}")
                        nc.scalar.dma_start(
                            bias_t[:], lbias[l][None, :].to_broadcast((128, H))
                        )
                        gbt = dict(g=g_t, b=b_t, bias=bias_t)

                    for t in range(TT):
                        hTt = hT[t]
                        ps = []
                        for i in range(NCH):
                            p = psY.tile(
                                [128, CH], F32, tag="psY", name=f"ps{l}_{t}_{i}"
                            )
                            ps.append(p)
                        for kt in range(KT):
                            if kt == KT // 2 and pending[1 - t] is not None:
                                pending[1 - t]()
                                pending[1 - t] = None
                            wt = w_half[kt // KH]
                            for i in range(NCH):
                                nc.tensor.matmul(
                                    ps[i][:],
                                    lhsT=hTt[:, kt, :],
                                    rhs=wt[:, kt % KH, i * CH : (i + 1) * CH],
                                    start=(kt == 0),
                                    stop=(kt == KT - 1),
                                    skip_group_check=True,
                                )
                        if l == L - 1 and t == 1:
                            # tile 0's final LN/cast/store runs in this
                            # unit's shadow; store0 lands mid-matmuls
                            finals[0] = emit_final(0)
                        z = zpool.tile([128, H], F32, tag="z", name=f"z{l}_{t}")
                        sums = smp.tile(
                            [128, 1 + NCH], F32, tag="sums", name=f"sm{l}_{t}"
                        )
                        resid = h_cur[t]
                        if use_gb:
                            hb = zpool.tile([128, H], F32, tag="hb", name=f"hb{l}_{t}")
                            nc.vector.tensor_tensor(
                                hb[:], h_cur[t][:], gbt["bias"][:], OP.add
                            )
                            resid = hb
                        for i in range(NCH):
                            nc.vector.tensor_add(
                                z[:, i * CH : (i + 1) * CH],
                                ps[i][:],
                                resid[:, i * CH : (i + 1) * CH],
                            )
                        nc.vector.tensor_reduce(
                            sums[:, 0:1], z[:], axis=AX.X, op=OP.add
                        )
                        for i in range(NCH):
                            nc.scalar.activation(
                                ps[i][:],
                                z[:, i * CH : (i + 1) * CH],
                                AF.Square,
                                accum_out=sums[:, 1 + i : 2 + i],
                            )
                        h_cur[t] = ln_finish(
                            z, sums[:, 0:1], sums[:, 1 : 1 + NCH],
                            gbt.get("g"), gbt.get("b"), f"{l}_{t}",
                        )
                        if l + 1 < L:
                            if l + 1 <= EARLY_PE:
                                # bootstrap barrier owns the DMA rings early
                                # on — transpose on the PE (slotted into the
                                # other tile's matmul stream via pending)
                                def mk(tt, ll, src):
                                    def emit():
                                        hT[tt] = pe_transpose(
                                            src, sigmas[ll + 1], hTp,
                                            f"{ll + 1}_{tt}",
                                        )
                                    return emit
                                pending[t] = mk(t, l, h_cur[t])
                            else:
                                # emitted NOW: the cast sits right behind
                                # this tile's LN in the vector FIFO and the
                                # XBAR runs during the other tile's matmuls
                                hT[t], _ = cast_transpose(
                                    h_cur[t], sigmas[l + 1], hTp,
                                    f"{l + 1}_{t}", t,
                                )
                        elif t == 1:
                            finals[1] = emit_final(1)

                # gathers: cc0 additionally waits for tile 1's XBAR — an
                # XBAR transpose issued while a collective owns the DMA
                # rings blocks until the collective completes
                for t in range(TT):
                    cc = nc.gpsimd.collective_compute(
                        "AllGather",
                        OP.bypass,
                        replica_groups=[list(range(NC))],
                        ins=[hT_loc[t][:]],
                        outs=[hT_all[t][:]],
                    )
                    tile.add_dep_helper(
                        cc.ins, finals[t][0].ins, sync=True,
                        reason=f"gather{t} waits on hT store{t}",
                    )
                    if t == 0:
                        tile.add_dep_helper(
                            cc.ins, finals[1][1].ins, sync=True,
                            reason="gather0 waits on tile1 XBAR (ring conflict)",
                        )
                    ccs[t] = cc

                # local head units overlap the AllGathers (psY banks reused)
                head_unit(hTfin[0], pid * TT + 0, "loc0", psY, "psY")
                head_unit(hTfin[1], pid * TT + 1, "loc1", psY, "psY")

            with ExitStack() as ctxB:
                peerp = ctxB.enter_context(tc.tile_pool(name="peer", bufs=2))
                psH = ctxB.enter_context(
                    tc.tile_pool(name="psH", bufs=NVC, space="PSUM")
                )

                # remote tokens: ring order c = (pid + r) & (NC-1)
                for r in range(1, NC):
                    c = (pid + r) & (NC - 1)
                    for t in range(TT):
                        pt = peerp.tile(
                            [128, KT, 128], FP16, tag="peer", name=f"peer{r}_{t}"
                        )
                        ld = nc.sync.dma_start(pt[:], hT_all[t][ts(c, 1), :, :])
                        tile.add_dep_helper(
                            ld.ins,
                            ccs[t].ins,
                            sync=True,
                            reason=f"peer load r{r} t{t} waits gather{t}",
                        )
                        head_unit(pt, c * TT + t, f"r{r}_{t}", psH, "ph")

    return nc


def _ternary(wmat):
    """Exact {-1,0,1} ternary tensor + fp32 scale, matching the reference."""
    w = np.asarray(wmat, dtype=np.float32)
    s = np.mean(np.abs(w), dtype=np.float32)
    t = np.clip(np.rint(w / (s + np.float32(1e-8))), -1.0, 1.0).astype(np.float32)
    return t, float(s)


def _split_scale(s):
    """s = sigma * 2^-e with sigma ~ 1 and 2^-e exact in fp8e4."""
    e = int(np.clip(np.round(-np.log2(s)), -7, 9))
    return s * (2.0**e), e


_NC_CACHE = {}
_LAST_RESULTS = None


def kernel(**inputs):
    global _LAST_RESULTS
    cfg = CFG_FULL
    L, H, NTOK, NC, TT, VS = (
        cfg["L"], cfg["H"], cfg["NTOK"], cfg["NC"], cfg["TT"], cfg["VS"],
    )
    KT = H // 128
    TPC = TT * 128  # tokens per core
    BF = ml_dtypes.bfloat16
    F8 = ml_dtypes.float8_e4m3fn

    ids = np.asarray(inputs["input_ids"]).astype(np.int64).reshape(-1)
    embed = np.asarray(inputs["embed"], dtype=np.float32)
    layer_w = np.asarray(inputs["layer_w"], dtype=np.float32)
    layer_b = np.asarray(inputs["layer_b"], dtype=np.float32)
    ln_g = np.asarray(inputs["ln_g"], dtype=np.float32)
    ln_b = np.asarray(inputs["ln_b"], dtype=np.float32)
    final_g = np.asarray(inputs["final_g"], dtype=np.float32)
    final_b = np.asarray(inputs["final_b"], dtype=np.float32)
    head_w = np.asarray(inputs["head_w"], dtype=np.float32)

    use_gb = not (
        np.all(layer_b == 0.0)
        and np.all(ln_g == 1.0)
        and np.all(ln_b == 0.0)
        and np.all(final_g == 1.0)
        and np.all(final_b == 0.0)
    )

    h0_full = embed[ids]  # [NTOK, H] fp32

    sigmas = []
    wT = np.empty([L, KT, 128, H], dtype=F8)
    for l in range(L):
        t, s = _ternary(layer_w[l])
        sig, e = _split_scale(s)
        sigmas.append(sig)
        wT[l] = (
            (np.ascontiguousarray(t.T) * np.float32(2.0**-e))
            .reshape(KT, 128, H)
            .astype(F8)
        )
    th, head_scale = _ternary(head_w)
    head_sigma, e_h = _split_scale(head_scale)
    headT = (
        (np.ascontiguousarray(th.T) * np.float32(2.0**-e_h))
        .reshape(KT, 128, -1)
        .astype(F8)
    )  # [KT, 128, V]

    key = (id(cfg), tuple(sigmas), head_sigma, use_gb)
    if key not in _NC_CACHE:
        _NC_CACHE.clear()
        nc = build_nc(cfg, sigmas, head_sigma, use_gb)
        # Bacc.finalize runs the TRN2 legalization passes (1-wait-per-
        # instruction event-semaphore split, matmul->ldweights wait motion,
        # register allocation). The PJRT exec path serializes nc as-is.
        nc.finalize()
        _NC_CACHE[key] = nc
    nc = _NC_CACHE[key]

    common = {
        "w": wT,
        "ident": np.eye(128, dtype=np.float32),
        "eps": np.full((128, 1), EPS, np.float32),
    }
    if use_gb:
        common.update(
            lng=ln_g.astype(BF),
            lnb=ln_b.astype(BF),
            lbias=layer_b.astype(BF),
            fing=final_g.astype(BF),
            finb=final_b.astype(BF),
        )
    in_maps = []
    for c in range(NC):
        h0c = np.ascontiguousarray(
            h0_full[c * TPC : (c + 1) * TPC].reshape(TT, 128, H)
        )
        # layer-0 transposed+scaled input, in XBAR block layout
        h0t = np.ascontiguousarray(
            (h0c * np.float32(sigmas[0]))
            .transpose(0, 2, 1)  # [TT, H, 128tok]
            .reshape(TT, KT, 128, 128)
            .transpose(0, 2, 1, 3)  # [TT, 128p, KT, 128tok]
        ).astype(np.float16)
        in_maps.append(
            dict(
                common,
                h0=h0c,
                h0T=h0t,
                hw=np.ascontiguousarray(headT[:, :, c * VS : (c + 1) * VS]),
            )
        )

    trace = bool(int(os.environ.get("TRIKERNEL_TRACE", "0")))
    res = run_bass_kernel_spmd(nc, in_maps, core_ids=list(range(NC)), trace=trace)
    _LAST_RESULTS = res

    full = np.concatenate(
        [np.asarray(res.results[c]["out"]) for c in range(NC)], axis=1
    )  # [NTOK, V] bf16
    return full.reshape(2, 1024, 32000).astype(np.float32)
